# revision 2
# baseline (speedup 1.0000x reference)
"""CaptionLoss (LSTM decode + cross-entropy) on 8 Trainium2 NeuronCores.

Strategy:
  - Host: build teacher-forced token ids, gather+transpose embedding rows,
    transpose weights into T-layout (feature on partition), cast matmul
    operands to bf16 (validated end-to-end rel err ~1e-7).
  - Device (one SPMD program on 8 cores):
      * ih-term precompute: ihT[2048, 3264] = W_ih @ X^T + bias  (PE)
      * 51-step LSTM in T-layout, h state stored as bf16 columns of
        hsT[512, 3264]; c state f32   (PE + ACT + DVE), replicated
      * fc/softmax partial: each core owns a 4000-wide vocab shard of
        fc_W; computes sum_v exp(h . w_v + b_v) per sequence position
        with ACT Exp + accum_out   (PE + DVE + ACT)
  - Host: combine per-core exp-sums, compute target logits from the
    exported hs (3264x512 dot rows), final loss reduction.
"""

import numpy as np
import ml_dtypes as mld

B = 64
T = 50
TP1 = T + 1
R = TP1 * B          # 3264 sequence rows, t-major (r = t*B + b)
H = 512
E = 512
G = 4 * H            # 2048 gate rows
V = 32000
NC = 8
VS = V // NC         # 4000 vocab shard
START_IDX = 1
STOP_IDX = 2
KC = H // 128        # 4 contraction chunks
MC_G = G // 128      # 16 gate row chunks
MC_R = (R + 127) // 128   # 26 row chunks (last has 64 valid rows)
NT_FC = 8            # vocab shard split into 8 chunks of 500
NV = VS // NT_FC     # 500

_BUILT = None


def _build():
    import concourse.bacc as bacc
    import concourse.mybir as mybir
    import concourse.tile as tile

    f32 = mybir.dt.float32
    bf16 = mybir.dt.bfloat16
    AF = mybir.ActivationFunctionType
    from concourse.alu_op_type import AluOpType

    nc = bacc.Bacc("TRN2", target_bir_lowering=False, debug=False,
                   num_devices=NC)

    # ---- DRAM I/O ----------------------------------------------------
    xTb_d = nc.dram_tensor("xTb", [H, B], bf16, kind="ExternalInput")
    xTf_d = nc.dram_tensor("xTf", [H, B], f32, kind="ExternalInput")
    XT_d = nc.dram_tensor("XT", [E, R], bf16, kind="ExternalInput")
    WihT_d = nc.dram_tensor("WihT", [E, G], bf16, kind="ExternalInput")
    WhhT_d = nc.dram_tensor("WhhT", [H, G], bf16, kind="ExternalInput")
    biasr_d = nc.dram_tensor("biasr", [128, MC_G], f32, kind="ExternalInput")
    fcWT_d = nc.dram_tensor("fcWT", [H, VS], bf16, kind="ExternalInput")
    fcb_d = nc.dram_tensor("fcb", [128, VS], f32, kind="ExternalInput")

    S_d = nc.dram_tensor("S", [128, MC_R], f32, kind="ExternalOutput")
    hs_d = nc.dram_tensor("hs", [128, KC * R], bf16, kind="ExternalOutput")

    with tile.TileContext(nc) as tc:
        with tc.tile_pool(name="glob", bufs=1) as gp:
            # ---- constants / state ----------------------------------
            WhhT = gp.tile([128, KC * G], bf16)
            nc.sync.dma_start(
                out=WhhT[:, :].rearrange("p (k g) -> p k g", k=KC),
                in_=WhhT_d.ap().rearrange("(k p) g -> p k g", p=128))
            biasr = gp.tile([128, MC_G], f32)
            nc.sync.dma_start(out=biasr[:, :], in_=biasr_d[:, :])
            xTb = gp.tile([128, KC * B], bf16)
            nc.sync.dma_start(
                out=xTb[:, :].rearrange("p (k b) -> p k b", k=KC),
                in_=xTb_d.ap().rearrange("(k p) b -> p k b", p=128))
            cT = gp.tile([128, KC * B], f32)
            nc.sync.dma_start(
                out=cT[:, :].rearrange("p (k b) -> p k b", k=KC),
                in_=xTf_d.ap().rearrange("(k p) b -> p k b", p=128))
            hsT = gp.tile([128, KC * R], bf16)

            with tc.tile_pool(name="lstm", bufs=1) as lp:
                ihT = lp.tile([128, MC_G * R], bf16)
                WihT = lp.tile([128, KC * G], bf16)
                nc.sync.dma_start(
                    out=WihT[:, :].rearrange("p (k g) -> p k g", k=KC),
                    in_=WihT_d.ap().rearrange("(k p) g -> p k g", p=128))

                # ---- phase B: ihT = W_ih @ X^T + bias ---------------
                with (tc.tile_pool(name="xs", bufs=3) as xsp,
                      tc.tile_pool(name="psB", bufs=2, space="PSUM") as psB):
                    n_chunks = []
                    c0 = 0
                    while c0 < R:
                        w = min(512, R - c0)
                        n_chunks.append((c0, w))
                        c0 += w
                    for (c0, w) in n_chunks:
                        xt = xsp.tile([128, KC * 512], bf16, tag="xt")
                        nc.sync.dma_start(
                            out=xt[:, 0:KC * w].rearrange(
                                "p (k n) -> p k n", k=KC),
                            in_=XT_d.ap().rearrange(
                                "(k p) n -> p k n", p=128)[:, :, c0:c0 + w])
                        for m in range(MC_G):
                            ps = psB.tile([128, 512], f32, tag="ps")
                            for k in range(KC):
                                nc.tensor.matmul(
                                    ps[:, 0:w],
                                    WihT[:, k * G + m * 128:
                                         k * G + (m + 1) * 128],
                                    xt[:, k * w:(k + 1) * w],
                                    start=(k == 0), stop=(k == KC - 1))
                            nc.vector.tensor_scalar_add(
                                out=ihT[:, m * R + c0:m * R + c0 + w],
                                in0=ps[:, 0:w],
                                scalar1=biasr[:, m:m + 1])

                # ---- phase C: LSTM scan -----------------------------
                with (tc.tile_pool(name="gs", bufs=2) as gsp,
                      tc.tile_pool(name="psC", bufs=4, space="PSUM") as psC):
                    ihT3 = ihT[:, :].rearrange("p (m r) -> p m r", m=MC_G)
                    for t in range(TP1):
                        if t == 0:
                            rhs = [xTb[:, k * B:(k + 1) * B] for k in range(KC)]
                        else:
                            rhs = [hsT[:, k * R + (t - 1) * B:
                                       k * R + t * B] for k in range(KC)]
                        ps0 = psC.tile([128, 512], f32, tag="ps0")
                        ps1 = psC.tile([128, 512], f32, tag="ps1")
                        for m in range(MC_G):
                            ps = ps0 if m < 8 else ps1
                            col = (m % 8) * B
                            for k in range(KC):
                                nc.tensor.matmul(
                                    ps[:, col:col + B],
                                    WhhT[:, k * G + m * 128:
                                         k * G + (m + 1) * 128],
                                    rhs[k],
                                    start=(k == 0), stop=(k == KC - 1))
                        g0 = gsp.tile([128, 512], f32, tag="g0")
                        g1 = gsp.tile([128, 512], f32, tag="g1")
                        nc.vector.tensor_tensor(
                            out=g0[:, :].rearrange("p (m b) -> p m b", m=8),
                            in0=ps0[:, :].rearrange("p (m b) -> p m b", m=8),
                            in1=ihT3[:, 0:8, t * B:(t + 1) * B],
                            op=AluOpType.add)
                        nc.vector.tensor_tensor(
                            out=g1[:, :].rearrange("p (m b) -> p m b", m=8),
                            in0=ps1[:, :].rearrange("p (m b) -> p m b", m=8),
                            in1=ihT3[:, 8:16, t * B:(t + 1) * B],
                            op=AluOpType.add)
                        sif = gsp.tile([128, 512], f32, tag="sif")
                        tg = gsp.tile([128, 256], f32, tag="tg")
                        so = gsp.tile([128, 256], f32, tag="so")
                        nc.scalar.activation(out=sif[:, :], in_=g0[:, :],
                                             func=AF.Sigmoid)
                        nc.scalar.activation(out=tg[:, :], in_=g1[:, 0:256],
                                             func=AF.Tanh)
                        nc.scalar.activation(out=so[:, :], in_=g1[:, 256:512],
                                             func=AF.Sigmoid)
                        ig = gsp.tile([128, 256], f32, tag="ig")
                        nc.vector.tensor_tensor(out=ig[:, :],
                                                in0=sif[:, 0:256],
                                                in1=tg[:, :],
                                                op=AluOpType.mult)
                        nc.vector.tensor_tensor(out=cT[:, :],
                                                in0=sif[:, 256:512],
                                                in1=cT[:, :],
                                                op=AluOpType.mult)
                        nc.vector.tensor_tensor(out=cT[:, :], in0=cT[:, :],
                                                in1=ig[:, :],
                                                op=AluOpType.add)
                        th = gsp.tile([128, 256], f32, tag="th")
                        nc.scalar.activation(out=th[:, :], in_=cT[:, :],
                                             func=AF.Tanh)
                        hout = bass_strided_h(hsT, t)
                        nc.vector.tensor_tensor(out=hout, in0=so[:, :],
                                                in1=th[:, :],
                                                op=AluOpType.mult)

            nc.sync.dma_start(out=hs_d[:, :], in_=hsT[:, :])

            # ---- phase D: fc + exp-sum over vocab shard -------------
            with (tc.tile_pool(name="fcp", bufs=1) as fp,
                  tc.tile_pool(name="fcs", bufs=3) as fsp,
                  tc.tile_pool(name="psD", bufs=2, space="PSUM") as psD):
                fcW = fp.tile([128, KC * VS], bf16)
                for k in range(KC):
                    nc.sync.dma_start(
                        out=fcW[:, k * VS:(k + 1) * VS],
                        in_=fcWT_d[k * 128:(k + 1) * 128, :])
                fcb = fp.tile([128, VS], f32)
                nc.sync.dma_start(out=fcb[:, :], in_=fcb_d[:, :])
                S_all = fp.tile([128, MC_R * NT_FC], f32)
                nc.vector.memset(S_all[:, :], 0.0)
                for n in range(NT_FC):
                    for m in range(MC_R):
                        mw = min(128, R - m * 128)
                        ps = psD.tile([128, NV], f32, tag="fps")
                        for k in range(KC):
                            nc.tensor.matmul(
                                ps[0:mw, :],
                                hsT[:, k * R + m * 128:k * R + m * 128 + mw],
                                fcW[:, k * VS + n * NV:k * VS + (n + 1) * NV],
                                start=(k == 0), stop=(k == KC - 1))
                        lg = fsp.tile([128, NV], f32, tag="lg")
                        nc.vector.tensor_tensor(
                            out=lg[0:mw, :], in0=ps[0:mw, :],
                            in1=fcb[0:mw, n * NV:(n + 1) * NV],
                            op=AluOpType.add)
                        eo = fsp.tile([128, NV], f32, tag="eo")
                        nc.scalar.activation(
                            out=eo[0:mw, :], in_=lg[0:mw, :], func=AF.Exp,
                            accum_out=S_all[0:mw, m * NT_FC + n:
                                            m * NT_FC + n + 1])
                S_fin = fp.tile([128, MC_R], f32)
                nc.vector.reduce_sum(
                    out=S_fin[:, :],
                    in_=S_all[:, :].rearrange("p (m n) -> p m n", n=NT_FC),
                    axis=mybir.AxisListType.X)
                nc.sync.dma_start(out=S_d[:, :], in_=S_fin[:, :])

    nc.compile()
    return nc


def bass_strided_h(hsT, t):
    # h output for step t: 4 chunks of 64 cols, strided by R in hsT
    return hsT[:, :].rearrange("p (k r) -> p k r", k=KC)[
        :, :, t * B:(t + 1) * B]


def _get_built():
    global _BUILT
    if _BUILT is None:
        _BUILT = _build()
    return _BUILT


def kernel(x, labels, emb, W_ih, W_hh, b_ih, b_hh, fc_W, fc_b):
    from concourse.bass_utils import run_bass_kernel_spmd

    x = np.asarray(x, np.float32)
    labels = np.asarray(labels)
    emb = np.asarray(emb, np.float32)
    W_ih = np.asarray(W_ih, np.float32)
    W_hh = np.asarray(W_hh, np.float32)
    b_ih = np.asarray(b_ih, np.float32)
    b_hh = np.asarray(b_hh, np.float32)
    fc_W = np.asarray(fc_W, np.float32)
    fc_b = np.asarray(fc_b, np.float32)

    lab = labels.astype(np.int64)
    inputs = np.concatenate(
        [np.full((B, 1), START_IDX, np.int64), lab], axis=1)      # [B, 51]
    targets = np.concatenate(
        [lab, np.full((B, 1), STOP_IDX, np.int64)], axis=1)       # [B, 51]
    idx = inputs.T.reshape(-1)      # [3264] t-major
    tgt = targets.T.reshape(-1)

    bf = mld.bfloat16
    xTb = np.ascontiguousarray(x.T).astype(bf)
    xTf = np.ascontiguousarray(x.T)
    XT = np.ascontiguousarray(emb[idx].T).astype(bf)              # [512, 3264]
    WihT = np.ascontiguousarray(W_ih.T).astype(bf)                # [512, 2048]
    WhhT = np.ascontiguousarray(W_hh.T).astype(bf)
    bias = (b_ih + b_hh).astype(np.float32)
    biasr = np.ascontiguousarray(bias.reshape(MC_G, 128).T)       # [128, 16]

    base = {"xTb": xTb, "xTf": xTf, "XT": XT, "WihT": WihT,
            "WhhT": WhhT, "biasr": biasr}
    in_maps = []
    for c in range(NC):
        sh = slice(c * VS, (c + 1) * VS)
        fcWT = np.ascontiguousarray(fc_W[sh].T).astype(bf)        # [512, 4000]
        fcb = np.ascontiguousarray(
            np.broadcast_to(fc_b[sh][None, :], (128, VS)))
        in_maps.append(dict(base, fcWT=fcWT, fcb=fcb))

    nc = _get_built()
    res = run_bass_kernel_spmd(nc, in_maps, core_ids=list(range(NC)))

    S_rows = np.zeros(R, np.float64)
    for c in range(NC):
        S_rows += np.asarray(
            res.results[c]["S"], np.float64).T.reshape(-1)[:R]
    hs0 = np.asarray(res.results[0]["hs"], np.float32)            # [128, 4*R]
    hs_rows = hs0.reshape(128, KC, R).transpose(2, 1, 0).reshape(R, H)
    Wt = fc_W[tgt].astype(bf).astype(np.float32)                  # [3264, 512]
    tgt_dot = (hs_rows * Wt).sum(1, dtype=np.float32)
    nll = np.log(S_rows) - (tgt_dot.astype(np.float64) + fc_b[tgt])
    loss = nll.sum() / B
    return np.float32(loss)


# revision 18
# speedup vs baseline: 6348.8368x; 6348.8368x over previous
"""CaptionLoss (LSTM decode + cross-entropy) on 8 Trainium2 NeuronCores.

Strategy:
  - Host: build teacher-forced token ids, gather+transpose embedding rows,
    transpose weights into T-layout (feature on partition). All matmul
    operands quantized to fp8 e4m3 with x16 scaling (loss rel err ~1e-6,
    validated against the jax reference in fp64-combined golden model).
  - Device (one SPMD program on 8 cores, no collectives):
      * phase B: ihT[2048, 3264] = 256*(W_ih @ X^T) + 256*bias, stored
        fp8 (x256) -- bulk matmul, amortized weight loads
      * phase C: 51-step LSTM in T-layout; gatesT = W_hh @ h^T + ih;
        h state stored fp8 (x16) as columns of hsT[512, 3264]; c f32.
        fp8 weights double PE weight-load rate (FWL)
      * phase D: per-core 4000-wide vocab shard of fc_W:
        S[r] = sum_v exp(h_r . w_v + b_v) via PE + DVE rescale+bias +
        ACT Exp with fused row-sum (accum_out)
      * B/C/D are emitted interleaved so fc and ih work fills PE stalls
        during the sequential LSTM chain.
  - Host: sum partial exp-sums across cores, target-logit dot from the
    exported hs, final log/sum reduction in f64.
"""

import numpy as np
import ml_dtypes as mld

B = 64
T = 50
TP1 = T + 1
R = TP1 * B          # 3264 sequence rows, t-major (r = t*B + b)
H = 512
E = 512
G = 4 * H            # 2048 gate rows
V = 32000
NC = 8
VS = V // NC         # 4000 vocab shard
START_IDX = 1
STOP_IDX = 2
KC = H // 128        # 4 contraction chunks
MC_G = G // 128      # 16 gate row chunks
MC_R = (R + 127) // 128   # 26 row chunks (last has 64 valid rows)
NT_FC = 8            # vocab shard split into 8 chunks (7x512 + 448)
NV = 512
SCL = 16.0           # fp8 operand scale; products carry 256x

_BUILT = None

import os
CFG_DBUDGET = int(os.environ.get("K_DBUDGET", "2"))
CFG_DEFER = int(os.environ.get("K_DEFER", "1"))
CFG_BCOPY = os.environ.get("K_BCOPY", "act")  # act|dve|split
CFG_PSX = int(os.environ.get("K_PSX", "5"))


def _build():
    import concourse.bacc as bacc
    import concourse.mybir as mybir
    import concourse.tile as tile

    f32 = mybir.dt.float32
    f8 = mybir.dt.float8e4
    bf16 = mybir.dt.bfloat16
    DR = mybir.MatmulPerfMode.DoubleRow
    AF = mybir.ActivationFunctionType
    from concourse.alu_op_type import AluOpType

    nc = bacc.Bacc("TRN2", target_bir_lowering=False, debug=False,
                   num_devices=NC)

    # ---- DRAM I/O (fp8 operands pre-scaled x16 by host) --------------
    xTb_d = nc.dram_tensor("xTb", [H, B], f8, kind="ExternalInput")
    xTf_d = nc.dram_tensor("xTf", [H, B], f32, kind="ExternalInput")
    XT_d = nc.dram_tensor("XT", [E, R], f8, kind="ExternalInput")
    WihT_d = nc.dram_tensor("WihT", [E, G], f8, kind="ExternalInput")
    WhhT_d = nc.dram_tensor("WhhT", [H, G], f8, kind="ExternalInput")
    biasr_d = nc.dram_tensor("biasr", [128, MC_G], f32, kind="ExternalInput")
    fcWT_d = nc.dram_tensor("fcWT", [H, VS], f8, kind="ExternalInput")
    fcb_d = nc.dram_tensor("fcb", [1, VS], bf16, kind="ExternalInput")

    S_d = nc.dram_tensor("S", [128, MC_R], f32, kind="ExternalOutput")
    hs_d = nc.dram_tensor("hs", [128, KC * R], f8, kind="ExternalOutput")

    with tile.TileContext(nc) as tc:
        with (tc.tile_pool(name="glob", bufs=1) as gp,
              tc.tile_pool(name="xs", bufs=3) as xsp,
              tc.tile_pool(name="gs", bufs=2) as gsp,
              tc.tile_pool(name="fcs", bufs=4) as fsp,
              tc.tile_pool(name="psX", bufs=CFG_PSX, space="PSUM") as psX,
              tc.tile_pool(name="psC", bufs=1, space="PSUM") as psC):
            # ---- constants / state ----------------------------------
            WhhT = gp.tile([128, KC * G], f8)
            nc.sync.dma_start(
                out=WhhT[:, :].rearrange("p (k g) -> p k g", k=KC),
                in_=WhhT_d.ap().rearrange("(k p) g -> p k g", p=128))
            WihT = gp.tile([128, KC * G], f8)
            nc.sync.dma_start(
                out=WihT[:, :].rearrange("p (k g) -> p k g", k=KC),
                in_=WihT_d.ap().rearrange("(k p) g -> p k g", p=128))
            biasr = gp.tile([128, MC_G], f32)
            nc.sync.dma_start(out=biasr[:, :], in_=biasr_d[:, :])
            xTb = gp.tile([128, KC * B], f8)
            nc.sync.dma_start(
                out=xTb[:, :].rearrange("p (k b) -> p k b", k=KC),
                in_=xTb_d.ap().rearrange("(k p) b -> p k b", p=128))
            cT = gp.tile([128, KC * B], f32)
            nc.sync.dma_start(
                out=cT[:, :].rearrange("p (k b) -> p k b", k=KC),
                in_=xTf_d.ap().rearrange("(k p) b -> p k b", p=128))
            fcW = gp.tile([128, KC * VS], f8)
            for k in range(KC):
                nc.sync.dma_start(
                    out=fcW[:, k * VS:(k + 1) * VS],
                    in_=fcWT_d[k * 128:(k + 1) * 128, :])
            fcb = gp.tile([1, VS], bf16)
            nc.sync.dma_start(out=fcb[:, :], in_=fcb_d[:, :])
            ones = gp.tile([1, 128], bf16)
            nc.gpsimd.memset(ones[:, :], 1.0)
            hsT = gp.tile([128, KC * R], f8)
            ihT = gp.tile([128, MC_G * R], f8)
            S_all = gp.tile([128, MC_R * NT_FC], f32)
            nc.vector.memset(S_all[:, :], 0.0)

            ihT3 = ihT[:, :].rearrange("p (m r) -> p m r", m=MC_G)

            # ---- phase B chunk: ihT cols [c0, c0+w) -----------------
            n_chunks = []
            c0 = 0
            while c0 < R:
                w = min(512, R - c0)
                n_chunks.append((c0, w))
                c0 += w

            def emit_B(j):
                c0, w = n_chunks[j]
                xt = xsp.tile([128, KC * 512], f8, tag="xt")
                nc.sync.dma_start(
                    out=xt[:, 0:KC * w].rearrange("p (k n) -> p k n", k=KC),
                    in_=XT_d.ap().rearrange(
                        "(k p) n -> p k n", p=128)[:, :, c0:c0 + w])
                xt3 = xt[:, 0:KC * w].rearrange("p (k n) -> p k n", k=KC)
                Wih3 = WihT[:, :].rearrange("p (k g) -> p k g", k=KC)
                for m in range(MC_G):
                    ps = psX.tile([128, 512], f32, tag="fps")
                    for pr in range(2):
                        nc.tensor.matmul(
                            ps[:, 0:w],
                            Wih3[:, 2 * pr:2 * pr + 2,
                                 m * 128:(m + 1) * 128],
                            xt3[:, 2 * pr:2 * pr + 2, :],
                            start=(pr == 0), stop=(pr == 1),
                            perf_mode=DR)
                    use_act = (CFG_BCOPY == "act" or
                               (CFG_BCOPY == "split" and m % 2 == 0))
                    if use_act:
                        nc.scalar.activation(
                            out=ihT[:, m * R + c0:m * R + c0 + w],
                            in_=ps[:, 0:w], func=AF.Identity,
                            bias=biasr[:, m:m + 1])
                    else:
                        nc.vector.tensor_scalar_add(
                            out=ihT[:, m * R + c0:m * R + c0 + w],
                            in0=ps[:, 0:w],
                            scalar1=biasr[:, m:m + 1])

            # ---- phase C step ---------------------------------------
            def emit_C(t):
                if t == 0:
                    rhs = [xTb[:, k * B:(k + 1) * B] for k in range(KC)]
                else:
                    rhs = [hsT[:, k * R + (t - 1) * B:k * R + t * B]
                           for k in range(KC)]
                # all-tanh gates (sigmoid(x) = (1+tanh(x/2))/2) so the whole
                # kernel stays on the exp_and_others ACT table; unified
                # tanh(x/512) via host x2 pre-scale of the g-gate rows.
                # g/o half (m 8..15) computed FIRST so its add/tanh overlap
                # the i/f half's matmuls.
                ps1 = psC.tile([128, 512], f32, tag="ps1")
                ps0 = psC.tile([128, 512], f32, tag="ps0")
                for m in list(range(8, 16)) + list(range(8)):
                    ps = ps0 if m < 8 else ps1
                    col = (m % 8) * B
                    for k in range(KC):
                        nc.tensor.matmul(
                            ps[:, col:col + B],
                            WhhT[:, k * G + m * 128:k * G + (m + 1) * 128],
                            rhs[k],
                            start=(k == 0), stop=(k == KC - 1))
                g1 = gsp.tile([128, 512], f32, tag="g1")
                nc.vector.tensor_tensor(
                    out=g1[:, :].rearrange("p (m b) -> p m b", m=8),
                    in0=ps1[:, :].rearrange("p (m b) -> p m b", m=8),
                    in1=ihT3[:, 8:16, t * B:(t + 1) * B],
                    op=AluOpType.add)
                s1 = gsp.tile([128, 512], f32, tag="s1")
                nc.scalar.activation(out=s1[:, :], in_=g1[:, :],
                                     func=AF.Tanh, scale=1.0 / 512)
                tg = s1[:, 0:256]
                to = s1[:, 256:512]
                to8 = gsp.tile([128, 256], f32, tag="to8")
                nc.vector.tensor_scalar(
                    out=to8[:, :], in0=to, scalar1=8.0, scalar2=8.0,
                    op0=AluOpType.mult, op1=AluOpType.add)
                g0 = gsp.tile([128, 512], f32, tag="g0")
                nc.vector.tensor_tensor(
                    out=g0[:, :].rearrange("p (m b) -> p m b", m=8),
                    in0=ps0[:, :].rearrange("p (m b) -> p m b", m=8),
                    in1=ihT3[:, 0:8, t * B:(t + 1) * B],
                    op=AluOpType.add)
                s0 = gsp.tile([128, 512], f32, tag="s0")
                nc.scalar.activation(out=s0[:, :], in_=g0[:, :],
                                     func=AF.Tanh, scale=1.0 / 512)
                ti = s0[:, 0:256]
                tf = s0[:, 256:512]
                # state is c2 = 2*c:  c2' = 0.5*(1+tf)*c2 + (1+ti)*tg
                u = gsp.tile([128, 256], f32, tag="u")
                nc.vector.scalar_tensor_tensor(
                    out=u[:, :], in0=tf, scalar=1.0,
                    in1=cT[:, :], op0=AluOpType.add, op1=AluOpType.mult)
                v = gsp.tile([128, 256], f32, tag="v")
                nc.vector.scalar_tensor_tensor(
                    out=v[:, :], in0=ti, scalar=1.0,
                    in1=tg, op0=AluOpType.add, op1=AluOpType.mult)
                nc.vector.scalar_tensor_tensor(
                    out=cT[:, :], in0=u[:, :], scalar=0.5,
                    in1=v[:, :], op0=AluOpType.mult, op1=AluOpType.add)
                th = gsp.tile([128, 256], f32, tag="th")
                nc.scalar.activation(out=th[:, :], in_=cT[:, :],
                                     func=AF.Tanh, scale=0.5)
                # h*16 = (8 + 8*to) * tanh(c), written as x16-scaled fp8
                hout = hsT[:, :].rearrange("p (k r) -> p k r", k=KC)[
                    :, :, t * B:(t + 1) * B]
                nc.vector.tensor_tensor(out=hout, in0=to8[:, :],
                                        in1=th[:, :], op=AluOpType.mult)

            # ---- phase D chunk: rows [128m, 128m+mw) ----------------
            hs3 = hsT[:, :].rearrange("p (k r) -> p k r", k=KC)
            fcW3 = fcW[:, :].rearrange("p (k v) -> p k v", k=KC)

            def emit_D_mm(m, n):
                mw = min(128, R - m * 128)
                nw = min(NV, VS - n * NV)
                ps = psX.tile([128, NV], f32, tag="fps")
                for pr in range(2):
                    nc.tensor.matmul(
                        ps[0:mw, 0:nw],
                        hs3[:, 2 * pr:2 * pr + 2,
                            m * 128:m * 128 + mw],
                        fcW3[:, 2 * pr:2 * pr + 2,
                             n * NV:n * NV + nw],
                        start=(pr == 0), stop=False, perf_mode=DR)
                nc.tensor.matmul(
                    ps[0:mw, 0:nw], ones[:, 0:mw],
                    fcb[:, n * NV:n * NV + nw],
                    start=False, stop=True)
                return (ps, m, n, mw, nw)

            def emit_D_exp(pend):
                ps, m, n, mw, nw = pend
                eo = fsp.tile([128, NV], bf16, tag="eo")
                nc.scalar.activation(
                    out=eo[0:mw, 0:nw], in_=ps[0:mw, 0:nw], func=AF.Exp,
                    scale=1.0 / 256,
                    accum_out=S_all[0:mw, m * NT_FC + n:
                                    m * NT_FC + n + 1])

            # ---- interleaved emission -------------------------------
            emit_B(0)
            emit_B(1)
            d_queue = [(m, n) for m in range(MC_R) for n in range(NT_FC)]
            d_next = 0
            pending = []
            for t in range(TP1):
                emit_C(t)
                # exps for last step's D tiles (psum long ready -> fills
                # ACT slack after th without blocking the gate chain)
                for p in pending:
                    emit_D_exp(p)
                pending = []
                if t % 8 == 0:
                    j = t // 8 + 2
                    if j < len(n_chunks):
                        emit_B(j)
                # new D matmul groups whose hs rows exist
                m_ready = (t - 1) // 2 if t >= 1 else -1
                n_emit = 0
                while n_emit < CFG_DBUDGET and d_next < len(d_queue):
                    m, n = d_queue[d_next]
                    if m > m_ready:
                        break
                    pending.append(emit_D_mm(m, n))
                    d_next += 1
                    n_emit += 1
                if not CFG_DEFER:
                    for p in pending:
                        emit_D_exp(p)
                    pending = []
            while d_next < len(d_queue) or pending:
                for p in pending:
                    emit_D_exp(p)
                pending = []
                n_emit = 0
                while n_emit < CFG_DBUDGET and d_next < len(d_queue):
                    m, n = d_queue[d_next]
                    pending.append(emit_D_mm(m, n))
                    d_next += 1
                    n_emit += 1
                if not CFG_DEFER:
                    for p in pending:
                        emit_D_exp(p)
                    pending = []

            nc.sync.dma_start(out=hs_d[:, :], in_=hsT[:, :])
            S_fin = gp.tile([128, MC_R], f32)
            nc.vector.reduce_sum(
                out=S_fin[:, :],
                in_=S_all[:, :].rearrange("p (m n) -> p m n", n=NT_FC),
                axis=mybir.AxisListType.X)
            nc.sync.dma_start(out=S_d[:, :], in_=S_fin[:, :])

    nc.compile()
    return nc


def _get_built():
    global _BUILT
    if _BUILT is None:
        _BUILT = _build()
    return _BUILT


def _q8(a):
    return np.clip(a, -240.0, 240.0).astype(mld.float8_e4m3)


def prep_in_maps(x, labels, emb, W_ih, W_hh, b_ih, b_hh, fc_W, fc_b):
    lab = labels.astype(np.int64)
    inputs = np.concatenate(
        [np.full((B, 1), START_IDX, np.int64), lab], axis=1)      # [B, 51]
    targets = np.concatenate(
        [lab, np.full((B, 1), STOP_IDX, np.int64)], axis=1)       # [B, 51]
    idx = inputs.T.reshape(-1)      # [3264] t-major
    tgt = targets.T.reshape(-1)

    # unified tanh(x/512): g-gate rows (the tanh gate) carry half scale
    gsc = np.ones((G, 1), np.float32)
    gsc[2 * H:3 * H] = 2.0
    base = {
        "xTb": _q8(np.ascontiguousarray(x.T) * SCL),
        "xTf": (np.ascontiguousarray(x.T) * 2.0).astype(np.float32),
        "XT": _q8(np.ascontiguousarray(emb[idx].T) * SCL),
        "WihT": _q8(np.ascontiguousarray((W_ih * gsc).T) * SCL),
        "WhhT": _q8(np.ascontiguousarray((W_hh * gsc).T) * SCL),
        "biasr": np.ascontiguousarray(
            ((b_ih + b_hh) * gsc[:, 0] * 256.0)
            .astype(np.float32).reshape(MC_G, 128).T),
    }
    in_maps = []
    for c in range(NC):
        sh = slice(c * VS, (c + 1) * VS)
        in_maps.append(dict(
            base,
            fcWT=_q8(np.ascontiguousarray(fc_W[sh].T) * SCL),
            fcb=(fc_b[sh][None, :] * 256.0).astype(mld.bfloat16)))
    return in_maps, tgt


def combine(results, tgt, fc_W, fc_b):
    S_rows = np.zeros(R, np.float64)
    for c in range(NC):
        S_rows += np.asarray(
            results[c]["S"], np.float64).T.reshape(-1)[:R]
    hs0 = np.asarray(results[0]["hs"]).astype(np.float32) / SCL   # [128, 4*R]
    hs_rows = hs0.reshape(128, KC, R).transpose(2, 1, 0).reshape(R, H)
    Wt = fc_W[tgt].astype(mld.bfloat16).astype(np.float32)        # [3264, 512]
    tgt_dot = (hs_rows * Wt).sum(1, dtype=np.float32)
    nll = np.log(S_rows) - (tgt_dot.astype(np.float64) + fc_b[tgt])
    return np.float32(nll.sum() / B)


def kernel(x, labels, emb, W_ih, W_hh, b_ih, b_hh, fc_W, fc_b):
    from concourse.bass_utils import run_bass_kernel_spmd

    x = np.asarray(x, np.float32)
    emb = np.asarray(emb, np.float32)
    W_ih = np.asarray(W_ih, np.float32)
    W_hh = np.asarray(W_hh, np.float32)
    b_ih = np.asarray(b_ih, np.float32)
    b_hh = np.asarray(b_hh, np.float32)
    fc_W = np.asarray(fc_W, np.float32)
    fc_b = np.asarray(fc_b, np.float32)

    in_maps, tgt = prep_in_maps(x, np.asarray(labels), emb, W_ih, W_hh,
                                b_ih, b_hh, fc_W, fc_b)
    nc = _get_built()
    res = run_bass_kernel_spmd(nc, in_maps, core_ids=list(range(NC)))
    return combine(res.results, tgt, fc_W, fc_b)


# revision 20
# speedup vs baseline: 6755.0224x; 1.0640x over previous
"""CaptionLoss (LSTM decode + cross-entropy) on 8 Trainium2 NeuronCores.

Strategy:
  - Host: build teacher-forced token ids, gather+transpose embedding rows,
    transpose weights into T-layout (feature on partition). All matmul
    operands quantized to fp8 e4m3 with x16 scaling (loss rel err ~1e-6,
    validated against the jax reference in fp64-combined golden model).
  - Device (one SPMD program on 8 cores, no collectives):
      * phase B: ihT[2048, 3264] = 256*(W_ih @ X^T) + 256*bias, stored
        fp8 (x256) -- bulk matmul, amortized weight loads
      * phase C: 51-step LSTM in T-layout; gatesT = W_hh @ h^T + ih;
        h state stored fp8 (x16) as columns of hsT[512, 3264]; c f32.
        fp8 weights double PE weight-load rate (FWL)
      * phase D: per-core 4000-wide vocab shard of fc_W:
        S[r] = sum_v exp(h_r . w_v + b_v): DoubleRow fp8 matmuls, the
        bias added in-PSUM by a K=1 ones-row matmul, then a single ACT
        Exp (scale fused) with accum_out row-sum per tile. The LSTM
        gates use an all-tanh formulation (sigmoid(x)=(1+tanh(x/2))/2,
        c-state stored as 2c) so every ACT op (Tanh/Exp/Identity) lives
        in one LUT table -- no 1.3us table reloads.
      * B/C/D are emitted interleaved (fc matmuls fill PE stalls during
        the sequential LSTM chain; their Exps are deferred one step so
        they land in ACT slack instead of delaying the gate chain).
  - Host: sum partial exp-sums across cores, target-logit dot from the
    exported hs, final log/sum reduction in f64.
"""

import numpy as np
import ml_dtypes as mld

B = 64
T = 50
TP1 = T + 1
R = TP1 * B          # 3264 sequence rows, t-major (r = t*B + b)
H = 512
E = 512
G = 4 * H            # 2048 gate rows
V = 32000
NC = 8
VS = V // NC         # 4000 vocab shard
START_IDX = 1
STOP_IDX = 2
KC = H // 128        # 4 contraction chunks
MC_G = G // 128      # 16 gate row chunks
MC_R = (R + 127) // 128   # 26 row chunks (last has 64 valid rows)
NT_FC = 8            # vocab shard split into 8 chunks (7x512 + 448)
NV = 512
SCL = 16.0           # fp8 operand scale; products carry 256x

_BUILT = None

import os
CFG_DBUDGET = int(os.environ.get("K_DBUDGET", "2"))
CFG_DEFER = int(os.environ.get("K_DEFER", "1"))
CFG_BCOPY = os.environ.get("K_BCOPY", "act")  # act|dve|split
CFG_PSX = int(os.environ.get("K_PSX", "5"))


def _build():
    import concourse.bacc as bacc
    import concourse.mybir as mybir
    import concourse.tile as tile

    f32 = mybir.dt.float32
    f8 = mybir.dt.float8e4
    bf16 = mybir.dt.bfloat16
    DR = mybir.MatmulPerfMode.DoubleRow
    AF = mybir.ActivationFunctionType
    from concourse.alu_op_type import AluOpType

    nc = bacc.Bacc("TRN2", target_bir_lowering=False, debug=False,
                   num_devices=NC)

    # ---- DRAM I/O (fp8 operands pre-scaled x16 by host) --------------
    xTb_d = nc.dram_tensor("xTb", [H, B], f8, kind="ExternalInput")
    xTf_d = nc.dram_tensor("xTf", [H, B], f32, kind="ExternalInput")
    XT_d = nc.dram_tensor("XT", [E, R], f8, kind="ExternalInput")
    WihT_d = nc.dram_tensor("WihT", [E, G], f8, kind="ExternalInput")
    WhhT_d = nc.dram_tensor("WhhT", [H, G], f8, kind="ExternalInput")
    biasr_d = nc.dram_tensor("biasr", [128, MC_G], f32, kind="ExternalInput")
    fcWT_d = nc.dram_tensor("fcWT", [H, VS], f8, kind="ExternalInput")
    fcb_d = nc.dram_tensor("fcb", [1, VS], bf16, kind="ExternalInput")

    S_d = nc.dram_tensor("S", [128, MC_R], f32, kind="ExternalOutput")
    hs_d = nc.dram_tensor("hs", [128, KC * R], f8, kind="ExternalOutput")

    with tile.TileContext(nc) as tc:
        with (tc.tile_pool(name="glob", bufs=1) as gp,
              tc.tile_pool(name="xs", bufs=3) as xsp,
              tc.tile_pool(name="gs", bufs=2) as gsp,
              tc.tile_pool(name="fcs", bufs=4) as fsp,
              tc.tile_pool(name="psX", bufs=CFG_PSX, space="PSUM") as psX,
              tc.tile_pool(name="psC", bufs=1, space="PSUM") as psC):
            # ---- constants / state ----------------------------------
            WhhT = gp.tile([128, KC * G], f8)
            nc.sync.dma_start(
                out=WhhT[:, :].rearrange("p (k g) -> p k g", k=KC),
                in_=WhhT_d.ap().rearrange("(k p) g -> p k g", p=128))
            WihT = gp.tile([128, KC * G], f8)
            nc.sync.dma_start(
                out=WihT[:, :].rearrange("p (k g) -> p k g", k=KC),
                in_=WihT_d.ap().rearrange("(k p) g -> p k g", p=128))
            biasr = gp.tile([128, MC_G], f32)
            nc.sync.dma_start(out=biasr[:, :], in_=biasr_d[:, :])
            xTb = gp.tile([128, KC * B], f8)
            nc.sync.dma_start(
                out=xTb[:, :].rearrange("p (k b) -> p k b", k=KC),
                in_=xTb_d.ap().rearrange("(k p) b -> p k b", p=128))
            cT = gp.tile([128, KC * B], f32)
            nc.sync.dma_start(
                out=cT[:, :].rearrange("p (k b) -> p k b", k=KC),
                in_=xTf_d.ap().rearrange("(k p) b -> p k b", p=128))
            fcW = gp.tile([128, KC * VS], f8)
            for k in range(KC):
                nc.sync.dma_start(
                    out=fcW[:, k * VS:(k + 1) * VS],
                    in_=fcWT_d[k * 128:(k + 1) * 128, :])
            fcb = gp.tile([1, VS], bf16)
            nc.sync.dma_start(out=fcb[:, :], in_=fcb_d[:, :])
            ones = gp.tile([1, 128], bf16)
            nc.gpsimd.memset(ones[:, :], 1.0)
            hsT = gp.tile([128, KC * R], f8)
            ihT = gp.tile([128, MC_G * R], f8)
            S_all = gp.tile([128, MC_R * NT_FC], f32)
            nc.vector.memset(S_all[:, :], 0.0)

            ihT3 = ihT[:, :].rearrange("p (m r) -> p m r", m=MC_G)

            # ---- phase B chunk: ihT cols [c0, c0+w) -----------------
            n_chunks = []
            c0 = 0
            while c0 < R:
                w = min(512, R - c0)
                n_chunks.append((c0, w))
                c0 += w

            Wih3 = WihT[:, :].rearrange("p (k g) -> p k g", k=KC)
            xt_tiles = {}

            def emit_B_dma(j):
                c0, w = n_chunks[j]
                xt = xsp.tile([128, KC * 512], f8, tag="xt")
                nc.sync.dma_start(
                    out=xt[:, 0:KC * w].rearrange("p (k n) -> p k n", k=KC),
                    in_=XT_d.ap().rearrange(
                        "(k p) n -> p k n", p=128)[:, :, c0:c0 + w])
                xt_tiles[j] = xt

            def emit_B_mm(j, m):
                c0, w = n_chunks[j]
                xt3 = xt_tiles[j][:, 0:KC * w].rearrange(
                    "p (k n) -> p k n", k=KC)
                ps = psX.tile([128, 512], f32, tag="fps")
                for pr in range(2):
                    nc.tensor.matmul(
                        ps[:, 0:w],
                        Wih3[:, 2 * pr:2 * pr + 2, m * 128:(m + 1) * 128],
                        xt3[:, 2 * pr:2 * pr + 2, :],
                        start=(pr == 0), stop=(pr == 1),
                        perf_mode=DR)
                return ("B", ps, j, m)

            def emit_B_copy(pend):
                _, ps, j, m = pend
                c0, w = n_chunks[j]
                use_act = (CFG_BCOPY == "act" or
                           (CFG_BCOPY == "split" and m % 2 == 0))
                if use_act:
                    nc.scalar.activation(
                        out=ihT[:, m * R + c0:m * R + c0 + w],
                        in_=ps[:, 0:w], func=AF.Identity,
                        bias=biasr[:, m:m + 1])
                else:
                    nc.vector.tensor_scalar_add(
                        out=ihT[:, m * R + c0:m * R + c0 + w],
                        in0=ps[:, 0:w],
                        scalar1=biasr[:, m:m + 1])

            # ---- phase C step ---------------------------------------
            def emit_C(t):
                if t == 0:
                    rhs = [xTb[:, k * B:(k + 1) * B] for k in range(KC)]
                else:
                    rhs = [hsT[:, k * R + (t - 1) * B:k * R + t * B]
                           for k in range(KC)]
                # all-tanh gates (sigmoid(x) = (1+tanh(x/2))/2) so the whole
                # kernel stays on the exp_and_others ACT table; unified
                # tanh(x/512) via host x2 pre-scale of the g-gate rows.
                # g/o half (m 8..15) computed FIRST so its add/tanh overlap
                # the i/f half's matmuls.
                ps1 = psC.tile([128, 512], f32, tag="ps1")
                ps0 = psC.tile([128, 512], f32, tag="ps0")
                for m in list(range(8, 16)) + list(range(8)):
                    ps = ps0 if m < 8 else ps1
                    col = (m % 8) * B
                    for k in range(KC):
                        nc.tensor.matmul(
                            ps[:, col:col + B],
                            WhhT[:, k * G + m * 128:k * G + (m + 1) * 128],
                            rhs[k],
                            start=(k == 0), stop=(k == KC - 1))
                g1 = gsp.tile([128, 512], f32, tag="g1")
                nc.vector.tensor_tensor(
                    out=g1[:, :].rearrange("p (m b) -> p m b", m=8),
                    in0=ps1[:, :].rearrange("p (m b) -> p m b", m=8),
                    in1=ihT3[:, 8:16, t * B:(t + 1) * B],
                    op=AluOpType.add)
                s1 = gsp.tile([128, 512], f32, tag="s1")
                nc.scalar.activation(out=s1[:, :], in_=g1[:, :],
                                     func=AF.Tanh, scale=1.0 / 512)
                tg = s1[:, 0:256]
                to = s1[:, 256:512]
                to8 = gsp.tile([128, 256], f32, tag="to8")
                nc.vector.tensor_scalar(
                    out=to8[:, :], in0=to, scalar1=8.0, scalar2=8.0,
                    op0=AluOpType.mult, op1=AluOpType.add)
                g0 = gsp.tile([128, 512], f32, tag="g0")
                nc.vector.tensor_tensor(
                    out=g0[:, :].rearrange("p (m b) -> p m b", m=8),
                    in0=ps0[:, :].rearrange("p (m b) -> p m b", m=8),
                    in1=ihT3[:, 0:8, t * B:(t + 1) * B],
                    op=AluOpType.add)
                s0 = gsp.tile([128, 512], f32, tag="s0")
                nc.scalar.activation(out=s0[:, :], in_=g0[:, :],
                                     func=AF.Tanh, scale=1.0 / 512)
                ti = s0[:, 0:256]
                tf = s0[:, 256:512]
                # state is c2 = 2*c:  c2' = 0.5*(1+tf)*c2 + (1+ti)*tg
                u = gsp.tile([128, 256], f32, tag="u")
                nc.vector.scalar_tensor_tensor(
                    out=u[:, :], in0=tf, scalar=1.0,
                    in1=cT[:, :], op0=AluOpType.add, op1=AluOpType.mult)
                v = gsp.tile([128, 256], f32, tag="v")
                nc.vector.scalar_tensor_tensor(
                    out=v[:, :], in0=ti, scalar=1.0,
                    in1=tg, op0=AluOpType.add, op1=AluOpType.mult)
                nc.vector.scalar_tensor_tensor(
                    out=cT[:, :], in0=u[:, :], scalar=0.5,
                    in1=v[:, :], op0=AluOpType.mult, op1=AluOpType.add)
                th = gsp.tile([128, 256], f32, tag="th")
                nc.scalar.activation(out=th[:, :], in_=cT[:, :],
                                     func=AF.Tanh, scale=0.5)
                # h*16 = (8 + 8*to) * tanh(c), written as x16-scaled fp8
                hout = hsT[:, :].rearrange("p (k r) -> p k r", k=KC)[
                    :, :, t * B:(t + 1) * B]
                nc.vector.tensor_tensor(out=hout, in0=to8[:, :],
                                        in1=th[:, :], op=AluOpType.mult)

            # ---- phase D chunk: rows [128m, 128m+mw) ----------------
            hs3 = hsT[:, :].rearrange("p (k r) -> p k r", k=KC)
            fcW3 = fcW[:, :].rearrange("p (k v) -> p k v", k=KC)

            def emit_D_mm(m, n):
                mw = min(128, R - m * 128)
                nw = min(NV, VS - n * NV)
                ps = psX.tile([128, NV], f32, tag="fps")
                for pr in range(2):
                    nc.tensor.matmul(
                        ps[0:mw, 0:nw],
                        hs3[:, 2 * pr:2 * pr + 2,
                            m * 128:m * 128 + mw],
                        fcW3[:, 2 * pr:2 * pr + 2,
                             n * NV:n * NV + nw],
                        start=(pr == 0), stop=False, perf_mode=DR)
                nc.tensor.matmul(
                    ps[0:mw, 0:nw], ones[:, 0:mw],
                    fcb[:, n * NV:n * NV + nw],
                    start=False, stop=True)
                return ("D", ps, m, n, mw, nw)

            def emit_D_exp(pend):
                _, ps, m, n, mw, nw = pend
                eo = fsp.tile([128, NV], bf16, tag="eo")
                nc.scalar.activation(
                    out=eo[0:mw, 0:nw], in_=ps[0:mw, 0:nw], func=AF.Exp,
                    scale=1.0 / 256,
                    accum_out=S_all[0:mw, m * NT_FC + n:
                                    m * NT_FC + n + 1])

            # ---- interleaved emission -------------------------------
            # chunks 0,1 of ihT are needed by the first LSTM steps: emit
            # their matmuls and copies up-front
            emit_B_dma(0)
            emit_B_dma(1)
            for j in (0, 1):
                for m in range(MC_G):
                    emit_B_copy(emit_B_mm(j, m))
            d_queue = [(m, n) for m in range(MC_R) for n in range(NT_FC)]
            d_next = 0
            b_queue = [(j, m) for j in range(2, len(n_chunks))
                       for m in range(MC_G)]
            b_next = 0
            pending = []
            for t in range(TP1):
                emit_C(t)
                # copies/exps for last step's tiles (psum long ready ->
                # fills engine slack without blocking the gate chain)
                for p in pending:
                    if p[0] == "B":
                        emit_B_copy(p)
                    else:
                        emit_D_exp(p)
                pending = []
                if t % 8 == 0:
                    j = t // 8 + 2
                    if j < len(n_chunks):
                        emit_B_dma(j)
                # 2 ih-precompute units per step (16 per 8-step chunk)
                n_emit = 0
                while n_emit < 2 and b_next < len(b_queue):
                    j, m = b_queue[b_next]
                    if j > t // 8 + 2:
                        break
                    pending.append(emit_B_mm(j, m))
                    b_next += 1
                    n_emit += 1
                # new D matmul groups whose hs rows exist
                m_ready = (t - 1) // 2 if t >= 1 else -1
                n_emit = 0
                while n_emit < CFG_DBUDGET and d_next < len(d_queue):
                    m, n = d_queue[d_next]
                    if m > m_ready:
                        break
                    pending.append(emit_D_mm(m, n))
                    d_next += 1
                    n_emit += 1
            while d_next < len(d_queue) or b_next < len(b_queue) or pending:
                for p in pending:
                    if p[0] == "B":
                        emit_B_copy(p)
                    else:
                        emit_D_exp(p)
                pending = []
                n_emit = 0
                while n_emit < 2 and b_next < len(b_queue):
                    j, m = b_queue[b_next]
                    pending.append(emit_B_mm(j, m))
                    b_next += 1
                    n_emit += 1
                n_emit = 0
                while n_emit < CFG_DBUDGET and d_next < len(d_queue):
                    m, n = d_queue[d_next]
                    pending.append(emit_D_mm(m, n))
                    d_next += 1
                    n_emit += 1

            nc.sync.dma_start(out=hs_d[:, :], in_=hsT[:, :])
            S_fin = gp.tile([128, MC_R], f32)
            nc.vector.reduce_sum(
                out=S_fin[:, :],
                in_=S_all[:, :].rearrange("p (m n) -> p m n", n=NT_FC),
                axis=mybir.AxisListType.X)
            nc.sync.dma_start(out=S_d[:, :], in_=S_fin[:, :])

    nc.compile()
    return nc


def _get_built():
    global _BUILT
    if _BUILT is None:
        _BUILT = _build()
    return _BUILT


def _q8(a):
    return np.clip(a, -240.0, 240.0).astype(mld.float8_e4m3)


def prep_in_maps(x, labels, emb, W_ih, W_hh, b_ih, b_hh, fc_W, fc_b):
    lab = labels.astype(np.int64)
    inputs = np.concatenate(
        [np.full((B, 1), START_IDX, np.int64), lab], axis=1)      # [B, 51]
    targets = np.concatenate(
        [lab, np.full((B, 1), STOP_IDX, np.int64)], axis=1)       # [B, 51]
    idx = inputs.T.reshape(-1)      # [3264] t-major
    tgt = targets.T.reshape(-1)

    # unified tanh(x/512): g-gate rows (the tanh gate) carry half scale
    gsc = np.ones((G, 1), np.float32)
    gsc[2 * H:3 * H] = 2.0
    base = {
        "xTb": _q8(np.ascontiguousarray(x.T) * SCL),
        "xTf": (np.ascontiguousarray(x.T) * 2.0).astype(np.float32),
        "XT": _q8(np.ascontiguousarray(emb[idx].T) * SCL),
        "WihT": _q8(np.ascontiguousarray((W_ih * gsc).T) * SCL),
        "WhhT": _q8(np.ascontiguousarray((W_hh * gsc).T) * SCL),
        "biasr": np.ascontiguousarray(
            ((b_ih + b_hh) * gsc[:, 0] * 256.0)
            .astype(np.float32).reshape(MC_G, 128).T),
    }
    in_maps = []
    for c in range(NC):
        sh = slice(c * VS, (c + 1) * VS)
        in_maps.append(dict(
            base,
            fcWT=_q8(np.ascontiguousarray(fc_W[sh].T) * SCL),
            fcb=(fc_b[sh][None, :] * 256.0).astype(mld.bfloat16)))
    return in_maps, tgt


def combine(results, tgt, fc_W, fc_b):
    S_rows = np.zeros(R, np.float64)
    for c in range(NC):
        S_rows += np.asarray(
            results[c]["S"], np.float64).T.reshape(-1)[:R]
    hs0 = np.asarray(results[0]["hs"]).astype(np.float32) / SCL   # [128, 4*R]
    hs_rows = hs0.reshape(128, KC, R).transpose(2, 1, 0).reshape(R, H)
    Wt = fc_W[tgt].astype(mld.bfloat16).astype(np.float32)        # [3264, 512]
    tgt_dot = (hs_rows * Wt).sum(1, dtype=np.float32)
    nll = np.log(S_rows) - (tgt_dot.astype(np.float64) + fc_b[tgt])
    return np.float32(nll.sum() / B)


def kernel(x, labels, emb, W_ih, W_hh, b_ih, b_hh, fc_W, fc_b):
    from concourse.bass_utils import run_bass_kernel_spmd

    x = np.asarray(x, np.float32)
    emb = np.asarray(emb, np.float32)
    W_ih = np.asarray(W_ih, np.float32)
    W_hh = np.asarray(W_hh, np.float32)
    b_ih = np.asarray(b_ih, np.float32)
    b_hh = np.asarray(b_hh, np.float32)
    fc_W = np.asarray(fc_W, np.float32)
    fc_b = np.asarray(fc_b, np.float32)

    in_maps, tgt = prep_in_maps(x, np.asarray(labels), emb, W_ih, W_hh,
                                b_ih, b_hh, fc_W, fc_b)
    nc = _get_built()
    res = run_bass_kernel_spmd(nc, in_maps, core_ids=list(range(NC)))
    return combine(res.results, tgt, fc_W, fc_b)
